# revision 23
# baseline (speedup 1.0000x reference)
"""MoE layer (top-1 switch routing) Trainium2 Bass kernel, expert-parallel over 8 cores.

Per core j (SPMD, same program, per-core data):
  Phase A: route its 1024-token slice (host-pretransposed xT, logits, softmax,
           argmax, in-slice cumsum via triangular matmul) -> per-token
           (idx, pos, gate) + per-slice stats -> table [1032,4] in DRAM.
  Phase B: AllGather tables across the 8 cores.
  Phase C: for all 8192 tokens compute this core's expert slot (global pos =
           slice pos + cross-slice carry, capacity clamp -> in-batch one-hot
           over 384 slots) and accumulate (token_id+1, gate) into PACKED
           per-m-tile PSUM via tiny permutation matmuls.  No indirect scatter.
  Phase D: indirect-gather the 1280 routed token rows of x, PE-transpose to
           xeT [d,1280] (f32r).
  Phase E: expert FFN y = gelu_tanh(xe@w1+b1)@w2+b2, f32r matmuls, H split in
           two halves with SBUF accumulation; epilogue scales rows by gate.
Host: inverse-permutation scatter of y rows into the output + aux/z losses
from the device-computed per-slice stats.
"""

import numpy as np

B, N, D, E, H = 4, 2048, 1024, 8, 4096
T = B * N                      # 8192 tokens
C = int(N * 1.25 // E)         # 320 expert capacity
NCORES = 8
SLICE = T // NCORES            # 1024 tokens routed per core
TT = SLICE // 128              # 8 token tiles per slice
M_TILES = (B * C) // 128       # 10 packed slot m-tiles (1280 rows)
TROWS = 1032                   # 1024 token rows + 5 stats rows + pad
S_PAD = 384                    # in-batch slot space (320 valid + trash at 383)
SLOT_ROWS = 1312

_COMPILED = None


def _pieces():
    """Packed-slot pieces: (b, cb0, m, p0, take): batch b's in-batch slots
    [cb0, cb0+take) land in packed m-tile m at partitions [p0, p0+take)."""
    out = []
    for b in range(B):
        c = 0
        while c < C:
            g = b * C + c
            m, p0 = divmod(g, 128)
            take = min(C - c, 128 - p0)
            out.append((b, c, m, p0, take))
            c += take
    return out


def _build():
    import concourse.bass as bass
    import concourse.bacc as bacc
    import concourse.mybir as mybir
    import concourse.tile as tile

    dt = mybir.dt
    AF = mybir.ActivationFunctionType
    OP = mybir.AluOpType

    nc = bacc.Bacc(trn_type="TRN2", num_devices=NCORES)

    xst_d = nc.dram_tensor("xst", [D, SLICE], dt.float32, kind="ExternalInput")
    xf_d = nc.dram_tensor("xf", [T, D], dt.float32, kind="ExternalInput")
    wr_d = nc.dram_tensor("wr", [D, E], dt.float32, kind="ExternalInput")
    w1_d = nc.dram_tensor("w1", [D, H], dt.float32r, kind="ExternalInput")
    w2_d = nc.dram_tensor("w2", [H, D], dt.float32r, kind="ExternalInput")
    b1t_d = nc.dram_tensor("b1t", [128, H // 128], dt.float32, kind="ExternalInput")
    b2_d = nc.dram_tensor("b2", [1, D], dt.float32, kind="ExternalInput")
    tri_d = nc.dram_tensor("tri", [128, 128], dt.float32, kind="ExternalInput")
    idn_d = nc.dram_tensor("idn", [128, 128], dt.float32, kind="ExternalInput")
    myj_d = nc.dram_tensor("myj", [128, 1], dt.float32, kind="ExternalInput")
    myeoh_d = nc.dram_tensor("myeoh", [1, E], dt.float32, kind="ExternalInput")

    yout_d = nc.dram_tensor("yout", [B * C, D], dt.float32, kind="ExternalOutput")
    slot_d = nc.dram_tensor("slot", [SLOT_ROWS, 2], dt.float32, kind="ExternalOutput")
    gstats_d = nc.dram_tensor("gstats", [NCORES, 20], dt.float32, kind="ExternalOutput")

    with tile.TileContext(nc) as tc:
        with tc.tile_pool(name="const", bufs=1) as cpool, \
             tc.tile_pool(name="dram", bufs=1, space="DRAM") as dpool:
            tri = cpool.tile([128, 128], dt.float32, tag="tri")
            idn = cpool.tile([128, 128], dt.float32, tag="idn")
            wr = cpool.tile([128, D // 128, E], dt.float32, tag="wr")
            myj = cpool.tile([128, 1], dt.float32, tag="myj")
            myeoh = cpool.tile([1, E], dt.float32, tag="myeoh")
            b1t = cpool.tile([128, H // 128], dt.float32, tag="b1t")
            b2r = cpool.tile([1, D], dt.float32, tag="b2r")
            ones_c = cpool.tile([128, 1], dt.float32, tag="ones_c")
            ones_r = cpool.tile([1, 128], dt.float32, tag="ones_r")
            iota_c = cpool.tile([128, 1], dt.float32, tag="iota_c")
            iota_e = cpool.tile([128, E], dt.float32, tag="iota_e")
            iota_s = cpool.tile([128, S_PAD], dt.float32, tag="iota_s")
            nc.sync.dma_start(out=tri[:], in_=tri_d[:])
            nc.sync.dma_start(out=idn[:], in_=idn_d[:])
            nc.sync.dma_start(out=wr[:], in_=wr_d[:].rearrange("(c p) e -> p c e", p=128))
            nc.sync.dma_start(out=myj[:], in_=myj_d[:])
            nc.sync.dma_start(out=myeoh[:], in_=myeoh_d[:])
            nc.sync.dma_start(out=b1t[:], in_=b1t_d[:])
            nc.sync.dma_start(out=b2r[:], in_=b2_d[:])
            nc.vector.memset(ones_c[:], 1.0)
            nc.vector.memset(ones_r[:], 1.0)
            ii = cpool.tile([128, 1], dt.int32, tag="ii")
            nc.gpsimd.iota(ii[:], pattern=[[0, 1]], base=0, channel_multiplier=1)
            nc.vector.tensor_copy(out=iota_c[:], in_=ii[:])
            ie = cpool.tile([128, E], dt.int32, tag="ie")
            nc.gpsimd.iota(ie[:], pattern=[[1, E]], base=0, channel_multiplier=0)
            nc.vector.tensor_copy(out=iota_e[:], in_=ie[:])
            iota_e3 = cpool.tile([128, 1, E], dt.float32, tag="iota_e3")
            nc.vector.tensor_copy(out=iota_e3[:], in_=iota_e[:])
            iota_em3 = cpool.tile([128, 1, E], dt.float32, tag="iota_em3")
            nc.vector.tensor_scalar_add(iota_em3[:], iota_e3[:], -100.0)
            it16 = cpool.tile([128, 2 * TT, 1], dt.int32, tag="it16")
            nc.gpsimd.iota(it16[:], pattern=[[1, 2 * TT], [0, 1]], base=0, channel_multiplier=0)
            ttg16f = cpool.tile([128, 2 * TT, 1], dt.float32, tag="ttg16f")
            nc.vector.tensor_copy(out=ttg16f[:], in_=it16[:])
            pb1 = cpool.tile([128, 2 * TT, 2], dt.bfloat16, tag="pb1")
            nc.vector.tensor_copy(out=pb1[:, :, 0:1],
                                  in_=iota_c[:, 0:1, None].to_broadcast([128, 2 * TT, 1]))
            nc.vector.memset(pb1[:, :, 1:2], 1.0)
            isl = cpool.tile([128, S_PAD], dt.int32, tag="isl")
            nc.gpsimd.iota(isl[:], pattern=[[1, S_PAD]], base=0, channel_multiplier=0)
            nc.vector.tensor_copy(out=iota_s[:], in_=isl[:])

            table = dpool.tile([TROWS, 4], dt.float32, tag="table")
            gtable = dpool.tile([NCORES * TROWS, 4], dt.float32, tag="gtable")

            # ---------------- Phase A: routing on own slice ----------------
            with tc.tile_pool(name="rsb", bufs=1) as rsb, \
                 tc.tile_pool(name="rps", bufs=2, space="PSUM") as rps, \
                 tc.tile_pool(name="rstat", bufs=1, space="PSUM") as rstat:
                xsT = rsb.tile([128, D // 128, SLICE], dt.float32, tag="xsT")
                for tt in range(TT):
                    nc.sync.dma_start(
                        out=xsT[:, :, tt * 128:(tt + 1) * 128],
                        in_=xst_d[:, tt * 128:(tt + 1) * 128].rearrange("(c p) t -> p c t", p=128))
                ps_tot = rstat.tile([1, E], dt.float32, space="PSUM", tag="ps_tot")
                ps_ps = rstat.tile([1, E], dt.float32, space="PSUM", tag="ps_ps")
                ps_z = rstat.tile([1, TT], dt.float32, space="PSUM", tag="ps_z")
                carry = rsb.tile([1, E], dt.float32, tag="carry")

                pl3 = rps.tile([128, TT, E], dt.float32, space="PSUM", tag="pl")
                for tt in range(TT):
                    for dd in range(D // 128):
                        nc.tensor.matmul(out=pl3[:, tt, :], lhsT=xsT[:, dd, tt * 128:(tt + 1) * 128],
                                         rhs=wr[:, dd, :], start=(dd == 0), stop=(dd == D // 128 - 1),
                                         skip_group_check=True)
                lg3 = rsb.tile([128, TT, E], dt.float32, tag="lg3")
                nc.vector.tensor_copy(out=lg3[:], in_=pl3[:])
                mx3 = rsb.tile([128, TT, 1], dt.float32, tag="mx3")
                nc.vector.tensor_reduce(out=mx3[:], in_=lg3[:], axis=mybir.AxisListType.X, op=OP.max)
                sh3 = rsb.tile([128, TT, E], dt.float32, tag="sh3")
                nc.vector.tensor_tensor(out=sh3[:], in0=lg3[:], in1=mx3[:].to_broadcast([128, TT, E]),
                                        op=OP.subtract)
                ex3 = rsb.tile([128, TT, E], dt.float32, tag="ex3")
                nc.scalar.activation(out=ex3[:], in_=sh3[:], func=AF.Exp)
                sume3 = rsb.tile([128, TT, 1], dt.float32, tag="sume3")
                nc.vector.tensor_reduce(out=sume3[:], in_=ex3[:], axis=mybir.AxisListType.X, op=OP.add)
                gate3 = rsb.tile([128, TT, 1], dt.float32, tag="gate3")
                nc.vector.reciprocal(gate3[:], sume3[:])
                lns3 = rsb.tile([128, TT, 1], dt.float32, tag="lns3")
                nc.scalar.activation(out=lns3[:], in_=sume3[:], func=AF.Ln)
                z3 = rsb.tile([128, TT, 1], dt.float32, tag="z3")
                nc.vector.tensor_add(z3[:], lns3[:], mx3[:])
                zsq3 = rsb.tile([128, TT, 1], dt.float32, tag="zsq3")
                nc.vector.tensor_mul(zsq3[:], z3[:], z3[:])
                mm3 = rsb.tile([128, TT, E], dt.float32, tag="mm3")
                nc.vector.tensor_tensor(out=mm3[:], in0=lg3[:], in1=mx3[:].to_broadcast([128, TT, E]),
                                        op=OP.is_equal)
                comb3 = rsb.tile([128, TT, E], dt.float32, tag="comb3")
                nc.vector.tensor_mul(comb3[:], mm3[:], iota_em3[:].to_broadcast([128, TT, E]))
                nc.vector.tensor_scalar_add(comb3[:], comb3[:], 100.0)
                idx3 = rsb.tile([128, TT, 1], dt.float32, tag="idx3")
                nc.vector.tensor_reduce(out=idx3[:], in_=comb3[:], axis=mybir.AxisListType.X, op=OP.min)
                oh3 = rsb.tile([128, TT, E], dt.float32, tag="oh3")
                nc.vector.tensor_tensor(out=oh3[:], in0=iota_e3[:].to_broadcast([128, TT, E]),
                                        in1=idx3[:].to_broadcast([128, TT, E]), op=OP.is_equal)
                probs3 = rsb.tile([128, TT, E], dt.float32, tag="probs3")
                nc.vector.tensor_mul(probs3[:], ex3[:], gate3[:].to_broadcast([128, TT, E]))

                pc3 = rps.tile([128, TT, E], dt.float32, space="PSUM", tag="pc")
                for tt in range(TT):
                    nc.tensor.matmul(out=pc3[:, tt, :], lhsT=tri[:], rhs=oh3[:, tt, :],
                                     start=True, stop=(tt == 0), skip_group_check=True)
                    if tt > 0:
                        nc.tensor.matmul(out=pc3[:, tt, :], lhsT=ones_r[:], rhs=carry[:],
                                         start=False, stop=True, skip_group_check=True)
                    nc.tensor.matmul(out=ps_tot[:], lhsT=ones_c[:], rhs=oh3[:, tt, :],
                                     start=(tt == 0), stop=(tt == TT - 1), skip_group_check=True)
                    if tt < TT - 1:
                        nc.vector.tensor_copy(out=carry[:], in_=ps_tot[:])
                for tt in range(TT):
                    nc.tensor.matmul(out=ps_ps[:], lhsT=ones_c[:], rhs=probs3[:, tt, :],
                                     start=(tt == 0), stop=(tt == TT - 1), skip_group_check=True)
                nc.tensor.matmul(out=ps_z[:], lhsT=ones_c[:], rhs=zsq3[:], start=True, stop=True)

                posw3 = rsb.tile([128, TT, E], dt.float32, tag="posw3")
                nc.vector.tensor_mul(posw3[:], pc3[:], oh3[:])
                pos3 = rsb.tile([128, TT, 1], dt.float32, tag="pos3")
                nc.vector.tensor_reduce(out=pos3[:], in_=posw3[:], axis=mybir.AxisListType.X, op=OP.add)
                nc.vector.tensor_scalar_add(pos3[:], pos3[:], -1.0)

                trow3 = rsb.tile([128, TT, 4], dt.float32, tag="trow3")
                nc.vector.tensor_copy(out=trow3[:, :, 0:1], in_=idx3[:])
                nc.vector.tensor_copy(out=trow3[:, :, 1:2], in_=pos3[:])
                nc.vector.tensor_copy(out=trow3[:, :, 2:3], in_=gate3[:])
                nc.vector.memset(trow3[:, :, 3:4], 0.0)
                nc.sync.dma_start(out=table[0:SLICE, :].rearrange("(t p) c -> p t c", p=128),
                                  in_=trow3[:])
                st = rsb.tile([1, 20], dt.float32, tag="st")
                nc.vector.tensor_copy(out=st[:, 0:8], in_=ps_tot[:])
                nc.vector.tensor_copy(out=st[:, 8:16], in_=ps_ps[:])
                zz8 = rsb.tile([1, TT], dt.float32, tag="zz8")
                nc.vector.tensor_copy(out=zz8[:], in_=ps_z[:])
                nc.vector.tensor_reduce(out=st[:, 16:17], in_=zz8[:], axis=mybir.AxisListType.X, op=OP.add)
                nc.vector.memset(st[:, 17:20], 0.0)
                tflat = table[:].rearrange("a b -> (a b)")
                nc.sync.dma_start(out=tflat[4096:4116], in_=st[:])

            # ---------------- Phase B: exchange ----------------
            nc.gpsimd.collective_compute(
                "AllGather", mybir.AluOpType.bypass,
                replica_groups=[list(range(NCORES))],
                ins=[table.opt()], outs=[gtable.opt()])

            gflat = gtable[:].rearrange("a b -> (a b)")
            with tc.tile_pool(name="gs", bufs=2) as gs:
                for s in range(NCORES):
                    gst = gs.tile([1, 20], dt.float32, tag="gst")
                    nc.sync.dma_start(out=gst[:], in_=gflat[s * 4128 + 4096: s * 4128 + 4116])
                    nc.sync.dma_start(out=gstats_d[s:s + 1, :], in_=gst[:])

            # ------------- Phase C: permutation into packed PSUM -------------
            pieces = _pieces()
            with tc.tile_pool(name="ffn", bufs=1) as ffn:
                xeT = [ffn.tile([128, B * C], dt.float32r, tag=f"xeT{d}", name=f"xeT{d}")
                       for d in range(D // 128)]
                gates = [ffn.tile([128, 1], dt.float32, tag=f"gate{m}", name=f"gate{m}")
                         for m in range(M_TILES)]
                ofsI = [ffn.tile([128, 1], dt.int32, tag=f"ofsI{m}", name=f"ofsI{m}")
                        for m in range(M_TILES)]
                pm = {}
                extracted = set()
                with tc.tile_pool(name="csb", bufs=2) as csb, \
                     tc.tile_pool(name="cps", bufs=1, space="PSUM") as cps, \
                     tc.tile_pool(name="pmp", bufs=1, space="PSUM") as pmp:
                    for b in range(B):
                        bp = [p for p in pieces if p[0] == b]
                        for (_, _, m, _, _) in bp:
                            if m not in pm:
                                pm[m] = pmp.tile([3, 128], dt.float32, space="PSUM",
                                                 tag=f"pm{m % 5}", name=f"pm{m}")
                        trw3b = csb.tile([128, 2 * TT, 4], dt.float32, tag="trw3b")
                        for si, s_ in enumerate((2 * b, 2 * b + 1)):
                            nc.sync.dma_start(
                                out=trw3b[:, si * TT:(si + 1) * TT, :],
                                in_=gtable[s_ * TROWS:s_ * TROWS + SLICE, :].rearrange(
                                    "(t p) c -> p t c", p=128))
                        cbc3 = csb.tile([128, 2 * TT, 1], dt.float32, tag="cbc3")
                        nc.vector.memset(cbc3[:, 0:TT, :], 0.0)
                        ce = csb.tile([1, E], dt.float32, tag="ce")
                        nc.sync.dma_start(out=ce[:],
                                          in_=gflat[2 * b * 4128 + 4096: 2 * b * 4128 + 4104])
                        cem = csb.tile([1, E], dt.float32, tag="cem")
                        nc.vector.tensor_mul(cem[:], ce[:], myeoh[:])
                        cnt = csb.tile([1, 1], dt.float32, tag="cnt")
                        nc.vector.tensor_reduce(out=cnt[:], in_=cem[:],
                                                axis=mybir.AxisListType.X, op=OP.add)
                        pcb = cps.tile([128, 1], dt.float32, space="PSUM", tag="pcb")
                        nc.tensor.matmul(out=pcb[:], lhsT=ones_r[:], rhs=cnt[:],
                                         start=True, stop=True)
                        cbv = csb.tile([128, 1], dt.float32, tag="cbv")
                        nc.vector.tensor_copy(out=cbv[:], in_=pcb[:])
                        nc.vector.tensor_copy(out=cbc3[:, TT:2 * TT, :],
                                              in_=cbv[:, 0:1, None].to_broadcast([128, TT, 1]))
                        mine3 = csb.tile([128, 2 * TT, 1], dt.float32, tag="mine3")
                        nc.vector.tensor_tensor(out=mine3[:], in0=trw3b[:, :, 0:1],
                                                in1=myj[:, 0:1, None].to_broadcast([128, 2 * TT, 1]),
                                                op=OP.is_equal)
                        posf3 = csb.tile([128, 2 * TT, 1], dt.float32, tag="posf3")
                        nc.vector.tensor_tensor(out=posf3[:], in0=trw3b[:, :, 1:2],
                                                in1=cbc3[:], op=OP.add)
                        inb3 = csb.tile([128, 2 * TT, 1], dt.float32, tag="inb3")
                        nc.vector.tensor_scalar(inb3[:], posf3[:], float(C),
                                                scalar2=None, op0=OP.is_lt)
                        sel3 = csb.tile([128, 2 * TT, 1], dt.float32, tag="sel3")
                        nc.vector.tensor_mul(sel3[:], mine3[:], inb3[:])
                        dstb3 = csb.tile([128, 2 * TT, 1], dt.float32, tag="dstb3")
                        nc.vector.tensor_scalar_add(dstb3[:], posf3[:], float(-(S_PAD - 1)))
                        nc.vector.tensor_mul(dstb3[:], dstb3[:], sel3[:])
                        nc.vector.tensor_scalar_add(dstb3[:], dstb3[:], float(S_PAD - 1))
                        ohs3 = csb.tile([128, 2 * TT, S_PAD], dt.bfloat16, tag="ohs3")
                        nc.vector.tensor_tensor(out=ohs3[:],
                                                in0=iota_s[:, None, :].to_broadcast([128, 2 * TT, S_PAD]),
                                                in1=dstb3[:].to_broadcast([128, 2 * TT, S_PAD]),
                                                op=OP.is_equal)
                        tokg3 = csb.tile([128, 2 * TT, 3], dt.bfloat16, tag="tokg3")
                        cdv = csb.tile([128, 2 * TT, 1], dt.float32, tag="cdv")
                        nc.vector.tensor_scalar_add(cdv[:], ttg16f[:], float(16 * b))
                        nc.vector.tensor_copy(out=tokg3[:, :, 0:1], in_=cdv[:])
                        nc.vector.tensor_copy(out=tokg3[:, :, 1:3], in_=pb1[:])
                        for ttg in range(2 * TT):
                            first = (ttg == 0)
                            last = (ttg == 2 * TT - 1)
                            for (_, cb0, m, p0, take) in bp:
                                nc.tensor.matmul(out=pm[m][:, p0:p0 + take],
                                                 lhsT=tokg3[:, ttg, :],
                                                 rhs=ohs3[:, ttg, cb0:cb0 + take],
                                                 start=first, stop=last,
                                                 skip_group_check=True)
                        # extract completed m-tiles
                        for m in sorted(pm.keys()):
                            if m in extracted:
                                continue
                            if (m + 1) * 128 <= (b + 1) * C:
                                pmv = csb.tile([3, 128], dt.float32, tag="pmv")
                                nc.vector.tensor_copy(out=pmv[:], in_=pm[m][:])
                                pmt2 = cps.tile([128, 3], dt.float32, space="PSUM", tag="pmt2")
                                nc.tensor.transpose(out=pmt2[:], in_=pmv[:],
                                                    identity=idn[0:3, 0:3])
                                slotv = csb.tile([128, 3], dt.float32, tag="slotv")
                                nc.vector.tensor_copy(out=slotv[:], in_=pmt2[:])
                                tokp1 = csb.tile([128, 1], dt.float32, tag="tokp1")
                                nc.vector.tensor_scalar_mul(tokp1[:], slotv[:, 0:1], 128.0)
                                nc.vector.tensor_add(tokp1[:], tokp1[:], slotv[:, 1:2])
                                nc.vector.tensor_add(tokp1[:], tokp1[:], slotv[:, 2:3])
                                tokf = csb.tile([128, 1], dt.float32, tag="tokf")
                                nc.vector.tensor_scalar_add(tokf[:], tokp1[:], -1.0)
                                nc.vector.tensor_copy(out=ofsI[m][:], in_=tokf[:])
                                # gather routing rows for the gate column
                                rsh = csb.tile([128, 1], dt.int32, tag="rsh")
                                nc.vector.tensor_scalar(rsh[:], ofsI[m][:], 10,
                                                        scalar2=None, op0=OP.arith_shift_right)
                                nc.vector.tensor_scalar(rsh[:], rsh[:], 8,
                                                        scalar2=None, op0=OP.mult)
                                grow = csb.tile([128, 1], dt.int32, tag="grow")
                                nc.vector.tensor_tensor(out=grow[:], in0=rsh[:], in1=ofsI[m][:],
                                                        op=OP.add)
                                gr4 = csb.tile([128, 4], dt.float32, tag="gr4")
                                nc.gpsimd.indirect_dma_start(
                                    out=gr4[:], out_offset=None, in_=gtable[:],
                                    in_offset=bass.IndirectOffsetOnAxis(ap=grow[:, 0:1], axis=0),
                                    bounds_check=NCORES * TROWS - 1, oob_is_err=False)
                                nc.vector.tensor_copy(out=gates[m][:], in_=gr4[:, 2:3])
                                slot2 = csb.tile([128, 2], dt.float32, tag="slot2")
                                nc.vector.tensor_copy(out=slot2[:, 0:1], in_=tokp1[:])
                                nc.vector.tensor_copy(out=slot2[:, 1:2], in_=gates[m][:])
                                nc.sync.dma_start(out=slot_d[m * 128:(m + 1) * 128, :], in_=slot2[:])
                                extracted.add(m)

                # ---------------- Phase D: gather + transpose ----------------
                with tc.tile_pool(name="dsb", bufs=3) as dsb, \
                     tc.tile_pool(name="dps", bufs=3, space="PSUM") as dps:
                    for m in range(M_TILES):
                        xe = dsb.tile([128, D], dt.float32, tag="xe")
                        nc.gpsimd.indirect_dma_start(
                            out=xe[:], out_offset=None, in_=xf_d[:],
                            in_offset=bass.IndirectOffsetOnAxis(ap=ofsI[m][:, 0:1], axis=0),
                            bounds_check=T - 1, oob_is_err=False)
                        for dd in range(D // 128):
                            pt = dps.tile([128, 128], dt.float32, space="PSUM", tag="sc")
                            nc.tensor.transpose(out=pt[:], in_=xe[:, dd * 128:(dd + 1) * 128],
                                                identity=idn[:])
                            nc.vector.tensor_copy(out=xeT[dd][:, m * 128:(m + 1) * 128], in_=pt[:])

                # ---------------- Phase E: FFN ----------------
                KH = 16                      # hh tiles per half
                _py_counter = [0]
                NCH = [(0, 512), (512, 512), (1024, 256)]
                y_acc = [ffn.tile([128, D], dt.float32, tag=f"yacc{m}", name=f"yacc{m}")
                         for m in range(M_TILES)]
                with tc.tile_pool(name="wsb", bufs=2) as wsb, \
                     tc.tile_pool(name="wp1", bufs=4) as wp1, \
                     tc.tile_pool(name="wp2", bufs=8) as wp2, \
                     tc.tile_pool(name="hsb", bufs=1) as hsb, \
                     tc.tile_pool(name="fps", bufs=3, space="PSUM") as fps, \
                     tc.tile_pool(name="lps", bufs=1, space="PSUM") as lps:
                    hT = [hsb.tile([128, B * C], dt.float32r, tag=f"hT{k}", name=f"hT{k}")
                          for k in range(KH)]
                    for half in range(2):
                        for hh in range(KH):
                            hg = half * KH + hh
                            w1t = wp1.tile([128, D // 128, 128], dt.float32r, tag="w1t")
                            nc.sync.dma_start(
                                out=w1t[:],
                                in_=w1_d[:, hg * 128:(hg + 1) * 128].rearrange("(c p) h -> p c h", p=128))
                            for (n0, nw) in NCH:
                                ph = fps.tile([128, 512], dt.float32, space="PSUM", tag="sc")
                                for kc in range(D // 128):
                                    nc.tensor.matmul(out=ph[:, :nw], lhsT=w1t[:, kc, :],
                                                     rhs=xeT[kc][:, n0:n0 + nw],
                                                     start=(kc == 0), stop=(kc == D // 128 - 1))
                                nc.scalar.activation(out=hT[hh][:, n0:n0 + nw], in_=ph[:, :nw],
                                                     func=AF.Gelu_apprx_tanh,
                                                     bias=b1t[:, hg:hg + 1], scale=1.0)
                        for mg in range(2):
                            ms = range(mg * 5, mg * 5 + 5)
                            for nh in range(2):
                                pys = {}
                                for m in ms:
                                    pys[m] = lps.tile([128, 512], dt.float32, space="PSUM",
                                                      tag=f"py{m % 5}", name=f"py{m % 5}")
                                for kk in range(KH):
                                    w2t = wp2.tile([128, 512], dt.float32r, tag="w2t")
                                    nc.sync.dma_start(
                                        out=w2t[:],
                                        in_=w2_d[(half * KH + kk) * 128:(half * KH + kk + 1) * 128,
                                                 nh * 512:(nh + 1) * 512])
                                    for m in ms:
                                        nc.tensor.matmul(out=pys[m][:],
                                                         lhsT=hT[kk][:, m * 128:(m + 1) * 128],
                                                         rhs=w2t[:], start=(kk == 0),
                                                         stop=(half == 0 and kk == KH - 1))
                                for m in ms:
                                    if half == 0:
                                        nc.vector.tensor_copy(out=y_acc[m][:, nh * 512:(nh + 1) * 512],
                                                              in_=pys[m][:])
                                    else:
                                        nc.tensor.matmul(out=pys[m][:], lhsT=ones_r[:],
                                                         rhs=b2r[:, nh * 512:(nh + 1) * 512],
                                                         start=False, stop=True)
                                        yf = wsb.tile([128, 512], dt.float32, tag="yf")
                                        nc.vector.tensor_add(yf[:], y_acc[m][:, nh * 512:(nh + 1) * 512],
                                                             pys[m][:])
                                        nc.vector.tensor_scalar_mul(yf[:], yf[:], gates[m][:, 0:1])
                                        nc.sync.dma_start(
                                            out=yout_d[m * 128:(m + 1) * 128, nh * 512:(nh + 1) * 512],
                                            in_=yf[:])

    nc.compile()
    return nc


def _prepare_inputs(inputs):
    x = np.asarray(inputs["x"], dtype=np.float32)
    wr = np.asarray(inputs["w_router"], dtype=np.float32)
    w1 = np.asarray(inputs["w1"], dtype=np.float32)
    b1 = np.asarray(inputs["b1"], dtype=np.float32)
    w2 = np.asarray(inputs["w2"], dtype=np.float32)
    b2 = np.asarray(inputs["b2"], dtype=np.float32)
    xflat = np.ascontiguousarray(x.reshape(T, D))
    tri = np.triu(np.ones((128, 128), dtype=np.float32))
    idn = np.eye(128, dtype=np.float32)
    in_maps = []
    for j in range(NCORES):
        myj = np.full((128, 1), float(j), dtype=np.float32)
        myeoh = np.zeros((1, E), dtype=np.float32)
        myeoh[0, j] = 1.0
        in_maps.append({
            "xst": np.ascontiguousarray(xflat[j * SLICE:(j + 1) * SLICE].T),
            "xf": xflat,
            "wr": wr,
            "w1": np.ascontiguousarray(w1[j]),
            "w2": np.ascontiguousarray(w2[j]),
            "b1t": np.ascontiguousarray(b1[j].reshape(H // 128, 128).T),
            "b2": b2[j].reshape(1, D),
            "tri": tri,
            "idn": idn,
            "myj": myj,
            "myeoh": myeoh,
        })
    return in_maps


def _assemble(results):
    out = np.zeros((T, D), dtype=np.float32)
    for j in range(NCORES):
        slot = results[j]["slot"]
        y = results[j]["yout"]
        tok = slot[:B * C, 0].astype(np.int64) - 1
        valid = tok >= 0
        out[tok[valid]] = y[valid]
    gs = results[0]["gstats"]          # [8, 20] identical on all cores
    counts = gs[:, 0:8]                # per slice per expert
    psums = gs[:, 8:16]
    z2 = gs[:, 16]
    density = (counts[0::2] + counts[1::2]) / float(N)       # [B, E]
    proxy = (psums[0::2] + psums[1::2]) / float(N)
    aux = np.float32(np.mean(np.sum(density * proxy, axis=-1)) * E * E)
    zl = np.float32(z2.sum() / float(T))
    return out.reshape(B, N, D), aux, zl


def kernel(**inputs):
    global _COMPILED
    from concourse.bass_utils import run_bass_kernel_spmd
    if _COMPILED is None:
        _COMPILED = _build()
    in_maps = _prepare_inputs(inputs)
    res = run_bass_kernel_spmd(_COMPILED, in_maps, list(range(NCORES)))
    return _assemble(res.results)


# revision 24
# speedup vs baseline: 1.0051x; 1.0051x over previous
"""MoE layer (top-1 switch routing) Trainium2 Bass kernel, expert-parallel over 8 cores.

Per core j (SPMD, same program, per-core data):
  Phase A: route its 1024-token slice (host-pretransposed xT, logits, softmax,
           argmax, in-slice cumsum via triangular matmul) -> per-token
           (idx, pos, gate) + per-slice stats -> table [1032,4] in DRAM.
  Phase B: AllGather tables across the 8 cores.
  Phase C: for all 8192 tokens compute this core's expert slot (global pos =
           slice pos + cross-slice carry, capacity clamp -> in-batch one-hot
           over 384 slots) and accumulate (token_id+1, gate) into PACKED
           per-m-tile PSUM via tiny permutation matmuls.  No indirect scatter.
  Phase D: indirect-gather the 1280 routed token rows of x, PE-transpose to
           xeT [d,1280] (f32r).
  Phase E: expert FFN y = gelu_tanh(xe@w1+b1)@w2+b2, f32r matmuls, H split in
           two halves with SBUF accumulation; epilogue scales rows by gate.
Host: inverse-permutation scatter of y rows into the output + aux/z losses
from the device-computed per-slice stats.
"""

import numpy as np

B, N, D, E, H = 4, 2048, 1024, 8, 4096
T = B * N                      # 8192 tokens
C = int(N * 1.25 // E)         # 320 expert capacity
NCORES = 8
SLICE = T // NCORES            # 1024 tokens routed per core
TT = SLICE // 128              # 8 token tiles per slice
M_TILES = (B * C) // 128       # 10 packed slot m-tiles (1280 rows)
TROWS = 1032                   # 1024 token rows + 5 stats rows + pad
S_PAD = 384                    # in-batch slot space (320 valid + trash at 383)
SLOT_ROWS = 1312

_COMPILED = None


def _pieces():
    """Packed-slot pieces: (b, cb0, m, p0, take): batch b's in-batch slots
    [cb0, cb0+take) land in packed m-tile m at partitions [p0, p0+take)."""
    out = []
    for b in range(B):
        c = 0
        while c < C:
            g = b * C + c
            m, p0 = divmod(g, 128)
            take = min(C - c, 128 - p0)
            out.append((b, c, m, p0, take))
            c += take
    return out


def _build():
    import concourse.bass as bass
    import concourse.bacc as bacc
    import concourse.mybir as mybir
    import concourse.tile as tile

    dt = mybir.dt
    AF = mybir.ActivationFunctionType
    OP = mybir.AluOpType

    nc = bacc.Bacc(trn_type="TRN2", num_devices=NCORES)

    xst_d = nc.dram_tensor("xst", [D, SLICE], dt.float32, kind="ExternalInput")
    xf_d = nc.dram_tensor("xf", [T, D], dt.float32, kind="ExternalInput")
    wr_d = nc.dram_tensor("wr", [D, E], dt.float32, kind="ExternalInput")
    w1_d = nc.dram_tensor("w1", [D, H], dt.float32r, kind="ExternalInput")
    w2_d = nc.dram_tensor("w2", [H, D], dt.float32r, kind="ExternalInput")
    b1t_d = nc.dram_tensor("b1t", [128, H // 128], dt.float32, kind="ExternalInput")
    b2_d = nc.dram_tensor("b2", [1, D], dt.float32, kind="ExternalInput")
    tri_d = nc.dram_tensor("tri", [128, 128], dt.float32, kind="ExternalInput")
    idn_d = nc.dram_tensor("idn", [128, 128], dt.float32, kind="ExternalInput")
    myj_d = nc.dram_tensor("myj", [128, 1], dt.float32, kind="ExternalInput")
    myeoh_d = nc.dram_tensor("myeoh", [1, E], dt.float32, kind="ExternalInput")

    yout_d = nc.dram_tensor("yout", [B * C, D], dt.float32, kind="ExternalOutput")
    slot_d = nc.dram_tensor("slot", [SLOT_ROWS, 2], dt.float32, kind="ExternalOutput")
    gstats_d = nc.dram_tensor("gstats", [NCORES, 20], dt.float32, kind="ExternalOutput")

    with tile.TileContext(nc) as tc:
        with tc.tile_pool(name="const", bufs=1) as cpool, \
             tc.tile_pool(name="dram", bufs=1, space="DRAM") as dpool:
            tri = cpool.tile([128, 128], dt.float32, tag="tri")
            idn = cpool.tile([128, 128], dt.float32, tag="idn")
            wr = cpool.tile([128, D // 128, E], dt.float32, tag="wr")
            myj = cpool.tile([128, 1], dt.float32, tag="myj")
            myeoh = cpool.tile([1, E], dt.float32, tag="myeoh")
            b1t = cpool.tile([128, H // 128], dt.float32, tag="b1t")
            b2r = cpool.tile([1, D], dt.float32, tag="b2r")
            ones_c = cpool.tile([128, 1], dt.float32, tag="ones_c")
            ones_r = cpool.tile([1, 128], dt.float32, tag="ones_r")
            iota_c = cpool.tile([128, 1], dt.float32, tag="iota_c")
            iota_e = cpool.tile([128, E], dt.float32, tag="iota_e")
            iota_s = cpool.tile([128, S_PAD], dt.float32, tag="iota_s")
            nc.sync.dma_start(out=tri[:], in_=tri_d[:])
            nc.sync.dma_start(out=idn[:], in_=idn_d[:])
            nc.sync.dma_start(out=wr[:], in_=wr_d[:].rearrange("(c p) e -> p c e", p=128))
            nc.sync.dma_start(out=myj[:], in_=myj_d[:])
            nc.sync.dma_start(out=myeoh[:], in_=myeoh_d[:])
            nc.sync.dma_start(out=b1t[:], in_=b1t_d[:])
            nc.sync.dma_start(out=b2r[:], in_=b2_d[:])
            nc.vector.memset(ones_c[:], 1.0)
            nc.vector.memset(ones_r[:], 1.0)
            ii = cpool.tile([128, 1], dt.int32, tag="ii")
            nc.gpsimd.iota(ii[:], pattern=[[0, 1]], base=0, channel_multiplier=1)
            nc.vector.tensor_copy(out=iota_c[:], in_=ii[:])
            ie = cpool.tile([128, E], dt.int32, tag="ie")
            nc.gpsimd.iota(ie[:], pattern=[[1, E]], base=0, channel_multiplier=0)
            nc.vector.tensor_copy(out=iota_e[:], in_=ie[:])
            iota_e3 = cpool.tile([128, 1, E], dt.float32, tag="iota_e3")
            nc.vector.tensor_copy(out=iota_e3[:], in_=iota_e[:])
            iota_em3 = cpool.tile([128, 1, E], dt.float32, tag="iota_em3")
            nc.vector.tensor_scalar_add(iota_em3[:], iota_e3[:], -100.0)
            it16 = cpool.tile([128, 2 * TT, 1], dt.int32, tag="it16")
            nc.gpsimd.iota(it16[:], pattern=[[1, 2 * TT], [0, 1]], base=0, channel_multiplier=0)
            ttg16f = cpool.tile([128, 2 * TT, 1], dt.float32, tag="ttg16f")
            nc.vector.tensor_copy(out=ttg16f[:], in_=it16[:])
            pb1 = cpool.tile([128, 2 * TT, 2], dt.bfloat16, tag="pb1")
            nc.vector.tensor_copy(out=pb1[:, :, 0:1],
                                  in_=iota_c[:, 0:1, None].to_broadcast([128, 2 * TT, 1]))
            nc.vector.memset(pb1[:, :, 1:2], 1.0)
            isl = cpool.tile([128, S_PAD], dt.int32, tag="isl")
            nc.gpsimd.iota(isl[:], pattern=[[1, S_PAD]], base=0, channel_multiplier=0)
            nc.vector.tensor_copy(out=iota_s[:], in_=isl[:])

            table = dpool.tile([TROWS, 4], dt.float32, tag="table")
            gtable = dpool.tile([NCORES * TROWS, 4], dt.float32, tag="gtable")

            # ---------------- Phase A: routing on own slice ----------------
            with tc.tile_pool(name="rsb", bufs=1) as rsb, \
                 tc.tile_pool(name="rps", bufs=2, space="PSUM") as rps, \
                 tc.tile_pool(name="rstat", bufs=1, space="PSUM") as rstat:
                xsT = rsb.tile([128, D // 128, SLICE], dt.float32, tag="xsT")
                for tt in range(TT):
                    nc.sync.dma_start(
                        out=xsT[:, :, tt * 128:(tt + 1) * 128],
                        in_=xst_d[:, tt * 128:(tt + 1) * 128].rearrange("(c p) t -> p c t", p=128))
                ps_tot = rstat.tile([1, E], dt.float32, space="PSUM", tag="ps_tot")
                ps_ps = rstat.tile([1, E], dt.float32, space="PSUM", tag="ps_ps")
                ps_z = rstat.tile([1, TT], dt.float32, space="PSUM", tag="ps_z")
                carry = rsb.tile([1, E], dt.float32, tag="carry")

                pl3 = rps.tile([128, TT, E], dt.float32, space="PSUM", tag="pl")
                for tt in range(TT):
                    for dd in range(D // 128):
                        nc.tensor.matmul(out=pl3[:, tt, :], lhsT=xsT[:, dd, tt * 128:(tt + 1) * 128],
                                         rhs=wr[:, dd, :], start=(dd == 0), stop=(dd == D // 128 - 1),
                                         skip_group_check=True)
                lg3 = rsb.tile([128, TT, E], dt.float32, tag="lg3")
                nc.vector.tensor_copy(out=lg3[:], in_=pl3[:])
                mx3 = rsb.tile([128, TT, 1], dt.float32, tag="mx3")
                nc.vector.tensor_reduce(out=mx3[:], in_=lg3[:], axis=mybir.AxisListType.X, op=OP.max)
                sh3 = rsb.tile([128, TT, E], dt.float32, tag="sh3")
                nc.vector.tensor_tensor(out=sh3[:], in0=lg3[:], in1=mx3[:].to_broadcast([128, TT, E]),
                                        op=OP.subtract)
                ex3 = rsb.tile([128, TT, E], dt.float32, tag="ex3")
                nc.scalar.activation(out=ex3[:], in_=sh3[:], func=AF.Exp)
                sume3 = rsb.tile([128, TT, 1], dt.float32, tag="sume3")
                nc.vector.tensor_reduce(out=sume3[:], in_=ex3[:], axis=mybir.AxisListType.X, op=OP.add)
                gate3 = rsb.tile([128, TT, 1], dt.float32, tag="gate3")
                nc.vector.reciprocal(gate3[:], sume3[:])
                lns3 = rsb.tile([128, TT, 1], dt.float32, tag="lns3")
                nc.scalar.activation(out=lns3[:], in_=sume3[:], func=AF.Ln)
                z3 = rsb.tile([128, TT, 1], dt.float32, tag="z3")
                nc.vector.tensor_add(z3[:], lns3[:], mx3[:])
                zsq3 = rsb.tile([128, TT, 1], dt.float32, tag="zsq3")
                nc.vector.tensor_mul(zsq3[:], z3[:], z3[:])
                mm3 = rsb.tile([128, TT, E], dt.float32, tag="mm3")
                nc.vector.tensor_tensor(out=mm3[:], in0=lg3[:], in1=mx3[:].to_broadcast([128, TT, E]),
                                        op=OP.is_equal)
                comb3 = rsb.tile([128, TT, E], dt.float32, tag="comb3")
                nc.vector.tensor_mul(comb3[:], mm3[:], iota_em3[:].to_broadcast([128, TT, E]))
                nc.vector.tensor_scalar_add(comb3[:], comb3[:], 100.0)
                idx3 = rsb.tile([128, TT, 1], dt.float32, tag="idx3")
                nc.vector.tensor_reduce(out=idx3[:], in_=comb3[:], axis=mybir.AxisListType.X, op=OP.min)
                oh3 = rsb.tile([128, TT, E], dt.float32, tag="oh3")
                nc.vector.tensor_tensor(out=oh3[:], in0=iota_e3[:].to_broadcast([128, TT, E]),
                                        in1=idx3[:].to_broadcast([128, TT, E]), op=OP.is_equal)
                probs3 = rsb.tile([128, TT, E], dt.float32, tag="probs3")
                nc.vector.tensor_mul(probs3[:], ex3[:], gate3[:].to_broadcast([128, TT, E]))

                pc3 = rps.tile([128, TT, E], dt.float32, space="PSUM", tag="pc")
                for tt in range(TT):
                    nc.tensor.matmul(out=pc3[:, tt, :], lhsT=tri[:], rhs=oh3[:, tt, :],
                                     start=True, stop=(tt == 0), skip_group_check=True)
                    if tt > 0:
                        nc.tensor.matmul(out=pc3[:, tt, :], lhsT=ones_r[:], rhs=carry[:],
                                         start=False, stop=True, skip_group_check=True)
                    nc.tensor.matmul(out=ps_tot[:], lhsT=ones_c[:], rhs=oh3[:, tt, :],
                                     start=(tt == 0), stop=(tt == TT - 1), skip_group_check=True)
                    if tt < TT - 1:
                        nc.vector.tensor_copy(out=carry[:], in_=ps_tot[:])
                for tt in range(TT):
                    nc.tensor.matmul(out=ps_ps[:], lhsT=ones_c[:], rhs=probs3[:, tt, :],
                                     start=(tt == 0), stop=(tt == TT - 1), skip_group_check=True)
                nc.tensor.matmul(out=ps_z[:], lhsT=ones_c[:], rhs=zsq3[:], start=True, stop=True)

                posw3 = rsb.tile([128, TT, E], dt.float32, tag="posw3")
                nc.vector.tensor_mul(posw3[:], pc3[:], oh3[:])
                pos3 = rsb.tile([128, TT, 1], dt.float32, tag="pos3")
                nc.vector.tensor_reduce(out=pos3[:], in_=posw3[:], axis=mybir.AxisListType.X, op=OP.add)
                nc.vector.tensor_scalar_add(pos3[:], pos3[:], -1.0)

                trow3 = rsb.tile([128, TT, 4], dt.float32, tag="trow3")
                nc.vector.tensor_copy(out=trow3[:, :, 0:1], in_=idx3[:])
                nc.vector.tensor_copy(out=trow3[:, :, 1:2], in_=pos3[:])
                nc.vector.tensor_copy(out=trow3[:, :, 2:3], in_=gate3[:])
                nc.vector.memset(trow3[:, :, 3:4], 0.0)
                nc.sync.dma_start(out=table[0:SLICE, :].rearrange("(t p) c -> p t c", p=128),
                                  in_=trow3[:])
                st = rsb.tile([1, 20], dt.float32, tag="st")
                nc.vector.tensor_copy(out=st[:, 0:8], in_=ps_tot[:])
                nc.vector.tensor_copy(out=st[:, 8:16], in_=ps_ps[:])
                zz8 = rsb.tile([1, TT], dt.float32, tag="zz8")
                nc.vector.tensor_copy(out=zz8[:], in_=ps_z[:])
                nc.vector.tensor_reduce(out=st[:, 16:17], in_=zz8[:], axis=mybir.AxisListType.X, op=OP.add)
                nc.vector.memset(st[:, 17:20], 0.0)
                tflat = table[:].rearrange("a b -> (a b)")
                nc.sync.dma_start(out=tflat[4096:4116], in_=st[:])

            # ---------------- Phase B: exchange ----------------
            nc.gpsimd.collective_compute(
                "AllGather", mybir.AluOpType.bypass,
                replica_groups=[list(range(NCORES))],
                ins=[table.opt()], outs=[gtable.opt()])

            gflat = gtable[:].rearrange("a b -> (a b)")
            with tc.tile_pool(name="gs", bufs=2) as gs:
                for s in range(NCORES):
                    gst = gs.tile([1, 20], dt.float32, tag="gst")
                    nc.sync.dma_start(out=gst[:], in_=gflat[s * 4128 + 4096: s * 4128 + 4116])
                    nc.sync.dma_start(out=gstats_d[s:s + 1, :], in_=gst[:])

            # ------------- Phase C: permutation into packed PSUM -------------
            pieces = _pieces()
            with tc.tile_pool(name="ffn", bufs=1) as ffn:
                xeT = [ffn.tile([128, B * C], dt.float32r, tag=f"xeT{d}", name=f"xeT{d}")
                       for d in range(D // 128)]
                gates = [ffn.tile([128, 1], dt.float32, tag=f"gate{m}", name=f"gate{m}")
                         for m in range(M_TILES)]
                ofsI = [ffn.tile([128, 1], dt.int32, tag=f"ofsI{m}", name=f"ofsI{m}")
                        for m in range(M_TILES)]
                pm = {}
                extracted = set()
                with tc.tile_pool(name="csb", bufs=3) as csb, \
                     tc.tile_pool(name="cps", bufs=1, space="PSUM") as cps, \
                     tc.tile_pool(name="pmp", bufs=1, space="PSUM") as pmp:
                    for b in range(B):
                        bp = [p for p in pieces if p[0] == b]
                        for (_, _, m, _, _) in bp:
                            if m not in pm:
                                pm[m] = pmp.tile([3, 128], dt.float32, space="PSUM",
                                                 tag=f"pm{m % 5}", name=f"pm{m}")
                        trw3b = csb.tile([128, 2 * TT, 4], dt.float32, tag="trw3b")
                        for si, s_ in enumerate((2 * b, 2 * b + 1)):
                            nc.sync.dma_start(
                                out=trw3b[:, si * TT:(si + 1) * TT, :],
                                in_=gtable[s_ * TROWS:s_ * TROWS + SLICE, :].rearrange(
                                    "(t p) c -> p t c", p=128))
                        cbc3 = csb.tile([128, 2 * TT, 1], dt.float32, tag="cbc3")
                        nc.vector.memset(cbc3[:, 0:TT, :], 0.0)
                        ce = csb.tile([1, E], dt.float32, tag="ce")
                        nc.sync.dma_start(out=ce[:],
                                          in_=gflat[2 * b * 4128 + 4096: 2 * b * 4128 + 4104])
                        cem = csb.tile([1, E], dt.float32, tag="cem")
                        nc.vector.tensor_mul(cem[:], ce[:], myeoh[:])
                        cnt = csb.tile([1, 1], dt.float32, tag="cnt")
                        nc.vector.tensor_reduce(out=cnt[:], in_=cem[:],
                                                axis=mybir.AxisListType.X, op=OP.add)
                        pcb = cps.tile([128, 1], dt.float32, space="PSUM", tag="pcb")
                        nc.tensor.matmul(out=pcb[:], lhsT=ones_r[:], rhs=cnt[:],
                                         start=True, stop=True)
                        cbv = csb.tile([128, 1], dt.float32, tag="cbv")
                        nc.vector.tensor_copy(out=cbv[:], in_=pcb[:])
                        nc.vector.tensor_copy(out=cbc3[:, TT:2 * TT, :],
                                              in_=cbv[:, 0:1, None].to_broadcast([128, TT, 1]))
                        mine3 = csb.tile([128, 2 * TT, 1], dt.float32, tag="mine3")
                        nc.vector.tensor_tensor(out=mine3[:], in0=trw3b[:, :, 0:1],
                                                in1=myj[:, 0:1, None].to_broadcast([128, 2 * TT, 1]),
                                                op=OP.is_equal)
                        posf3 = csb.tile([128, 2 * TT, 1], dt.float32, tag="posf3")
                        nc.vector.tensor_tensor(out=posf3[:], in0=trw3b[:, :, 1:2],
                                                in1=cbc3[:], op=OP.add)
                        inb3 = csb.tile([128, 2 * TT, 1], dt.float32, tag="inb3")
                        nc.vector.tensor_scalar(inb3[:], posf3[:], float(C),
                                                scalar2=None, op0=OP.is_lt)
                        sel3 = csb.tile([128, 2 * TT, 1], dt.float32, tag="sel3")
                        nc.vector.tensor_mul(sel3[:], mine3[:], inb3[:])
                        dstb3 = csb.tile([128, 2 * TT, 1], dt.float32, tag="dstb3")
                        nc.vector.tensor_scalar_add(dstb3[:], posf3[:], float(-(S_PAD - 1)))
                        nc.vector.tensor_mul(dstb3[:], dstb3[:], sel3[:])
                        nc.vector.tensor_scalar_add(dstb3[:], dstb3[:], float(S_PAD - 1))
                        ohs3 = csb.tile([128, 2 * TT, S_PAD], dt.bfloat16, tag="ohs3")
                        nc.vector.tensor_tensor(out=ohs3[:],
                                                in0=iota_s[:, None, :].to_broadcast([128, 2 * TT, S_PAD]),
                                                in1=dstb3[:].to_broadcast([128, 2 * TT, S_PAD]),
                                                op=OP.is_equal)
                        tokg3 = csb.tile([128, 2 * TT, 3], dt.bfloat16, tag="tokg3")
                        cdv = csb.tile([128, 2 * TT, 1], dt.float32, tag="cdv")
                        nc.vector.tensor_scalar_add(cdv[:], ttg16f[:], float(16 * b))
                        nc.vector.tensor_copy(out=tokg3[:, :, 0:1], in_=cdv[:])
                        nc.vector.tensor_copy(out=tokg3[:, :, 1:3], in_=pb1[:])
                        for ttg in range(2 * TT):
                            first = (ttg == 0)
                            last = (ttg == 2 * TT - 1)
                            for (_, cb0, m, p0, take) in bp:
                                nc.tensor.matmul(out=pm[m][:, p0:p0 + take],
                                                 lhsT=tokg3[:, ttg, :],
                                                 rhs=ohs3[:, ttg, cb0:cb0 + take],
                                                 start=first, stop=last,
                                                 skip_group_check=True)
                        # extract completed m-tiles
                        for m in sorted(pm.keys()):
                            if m in extracted:
                                continue
                            if (m + 1) * 128 <= (b + 1) * C:
                                pmv = csb.tile([3, 128], dt.float32, tag="pmv")
                                nc.vector.tensor_copy(out=pmv[:], in_=pm[m][:])
                                pmt2 = cps.tile([128, 3], dt.float32, space="PSUM", tag="pmt2")
                                nc.tensor.transpose(out=pmt2[:], in_=pmv[:],
                                                    identity=idn[0:3, 0:3])
                                slotv = csb.tile([128, 3], dt.float32, tag="slotv")
                                nc.vector.tensor_copy(out=slotv[:], in_=pmt2[:])
                                tokp1 = csb.tile([128, 1], dt.float32, tag="tokp1")
                                nc.vector.tensor_scalar_mul(tokp1[:], slotv[:, 0:1], 128.0)
                                nc.vector.tensor_add(tokp1[:], tokp1[:], slotv[:, 1:2])
                                nc.vector.tensor_add(tokp1[:], tokp1[:], slotv[:, 2:3])
                                tokf = csb.tile([128, 1], dt.float32, tag="tokf")
                                nc.vector.tensor_scalar_add(tokf[:], tokp1[:], -1.0)
                                nc.vector.tensor_copy(out=ofsI[m][:], in_=tokf[:])
                                # gather routing rows for the gate column
                                rsh = csb.tile([128, 1], dt.int32, tag="rsh")
                                nc.vector.tensor_scalar(rsh[:], ofsI[m][:], 10,
                                                        scalar2=None, op0=OP.arith_shift_right)
                                nc.vector.tensor_scalar(rsh[:], rsh[:], 8,
                                                        scalar2=None, op0=OP.mult)
                                grow = csb.tile([128, 1], dt.int32, tag="grow")
                                nc.vector.tensor_tensor(out=grow[:], in0=rsh[:], in1=ofsI[m][:],
                                                        op=OP.add)
                                gr4 = csb.tile([128, 4], dt.float32, tag="gr4")
                                nc.gpsimd.indirect_dma_start(
                                    out=gr4[:], out_offset=None, in_=gtable[:],
                                    in_offset=bass.IndirectOffsetOnAxis(ap=grow[:, 0:1], axis=0),
                                    bounds_check=NCORES * TROWS - 1, oob_is_err=False)
                                nc.vector.tensor_copy(out=gates[m][:], in_=gr4[:, 2:3])
                                slot2 = csb.tile([128, 2], dt.float32, tag="slot2")
                                nc.vector.tensor_copy(out=slot2[:, 0:1], in_=tokp1[:])
                                nc.vector.tensor_copy(out=slot2[:, 1:2], in_=gates[m][:])
                                nc.sync.dma_start(out=slot_d[m * 128:(m + 1) * 128, :], in_=slot2[:])
                                extracted.add(m)

                # ---------------- Phase D: gather + transpose ----------------
                with tc.tile_pool(name="dsb", bufs=4) as dsb, \
                     tc.tile_pool(name="dps", bufs=3, space="PSUM") as dps:
                    for m in range(M_TILES):
                        xe = dsb.tile([128, D], dt.float32, tag="xe")
                        nc.gpsimd.indirect_dma_start(
                            out=xe[:], out_offset=None, in_=xf_d[:],
                            in_offset=bass.IndirectOffsetOnAxis(ap=ofsI[m][:, 0:1], axis=0),
                            bounds_check=T - 1, oob_is_err=False)
                        for dd in range(D // 128):
                            pt = dps.tile([128, 128], dt.float32, space="PSUM", tag="sc")
                            nc.tensor.transpose(out=pt[:], in_=xe[:, dd * 128:(dd + 1) * 128],
                                                identity=idn[:])
                            nc.vector.tensor_copy(out=xeT[dd][:, m * 128:(m + 1) * 128], in_=pt[:])

                # ---------------- Phase E: FFN ----------------
                KH = 16                      # hh tiles per half
                _py_counter = [0]
                NCH = [(0, 512), (512, 512), (1024, 256)]
                y_acc = [ffn.tile([128, D], dt.float32, tag=f"yacc{m}", name=f"yacc{m}")
                         for m in range(M_TILES)]
                with tc.tile_pool(name="wsb", bufs=2) as wsb, \
                     tc.tile_pool(name="wp1", bufs=4) as wp1, \
                     tc.tile_pool(name="wp2", bufs=8) as wp2, \
                     tc.tile_pool(name="hsb", bufs=1) as hsb, \
                     tc.tile_pool(name="fps", bufs=3, space="PSUM") as fps, \
                     tc.tile_pool(name="lps", bufs=1, space="PSUM") as lps:
                    hT = [hsb.tile([128, B * C], dt.float32r, tag=f"hT{k}", name=f"hT{k}")
                          for k in range(KH)]
                    for half in range(2):
                        for hh in range(KH):
                            hg = half * KH + hh
                            w1t = wp1.tile([128, D // 128, 128], dt.float32r, tag="w1t")
                            nc.sync.dma_start(
                                out=w1t[:],
                                in_=w1_d[:, hg * 128:(hg + 1) * 128].rearrange("(c p) h -> p c h", p=128))
                            for (n0, nw) in NCH:
                                ph = fps.tile([128, 512], dt.float32, space="PSUM", tag="sc")
                                for kc in range(D // 128):
                                    nc.tensor.matmul(out=ph[:, :nw], lhsT=w1t[:, kc, :],
                                                     rhs=xeT[kc][:, n0:n0 + nw],
                                                     start=(kc == 0), stop=(kc == D // 128 - 1))
                                nc.scalar.activation(out=hT[hh][:, n0:n0 + nw], in_=ph[:, :nw],
                                                     func=AF.Gelu_apprx_tanh,
                                                     bias=b1t[:, hg:hg + 1], scale=1.0)
                        for mg in range(2):
                            ms = range(mg * 5, mg * 5 + 5)
                            for nh in range(2):
                                pys = {}
                                for m in ms:
                                    pys[m] = lps.tile([128, 512], dt.float32, space="PSUM",
                                                      tag=f"py{m % 5}", name=f"py{m % 5}")
                                for kk in range(KH):
                                    w2t = wp2.tile([128, 512], dt.float32r, tag="w2t")
                                    nc.sync.dma_start(
                                        out=w2t[:],
                                        in_=w2_d[(half * KH + kk) * 128:(half * KH + kk + 1) * 128,
                                                 nh * 512:(nh + 1) * 512])
                                    for m in ms:
                                        nc.tensor.matmul(out=pys[m][:],
                                                         lhsT=hT[kk][:, m * 128:(m + 1) * 128],
                                                         rhs=w2t[:], start=(kk == 0),
                                                         stop=(half == 0 and kk == KH - 1))
                                for m in ms:
                                    if half == 0:
                                        nc.vector.tensor_copy(out=y_acc[m][:, nh * 512:(nh + 1) * 512],
                                                              in_=pys[m][:])
                                    else:
                                        nc.tensor.matmul(out=pys[m][:], lhsT=ones_r[:],
                                                         rhs=b2r[:, nh * 512:(nh + 1) * 512],
                                                         start=False, stop=True)
                                        yf = wsb.tile([128, 512], dt.float32, tag="yf")
                                        nc.vector.tensor_add(yf[:], y_acc[m][:, nh * 512:(nh + 1) * 512],
                                                             pys[m][:])
                                        nc.vector.tensor_scalar_mul(yf[:], yf[:], gates[m][:, 0:1])
                                        nc.sync.dma_start(
                                            out=yout_d[m * 128:(m + 1) * 128, nh * 512:(nh + 1) * 512],
                                            in_=yf[:])

    nc.compile()
    return nc


def _prepare_inputs(inputs):
    x = np.asarray(inputs["x"], dtype=np.float32)
    wr = np.asarray(inputs["w_router"], dtype=np.float32)
    w1 = np.asarray(inputs["w1"], dtype=np.float32)
    b1 = np.asarray(inputs["b1"], dtype=np.float32)
    w2 = np.asarray(inputs["w2"], dtype=np.float32)
    b2 = np.asarray(inputs["b2"], dtype=np.float32)
    xflat = np.ascontiguousarray(x.reshape(T, D))
    tri = np.triu(np.ones((128, 128), dtype=np.float32))
    idn = np.eye(128, dtype=np.float32)
    in_maps = []
    for j in range(NCORES):
        myj = np.full((128, 1), float(j), dtype=np.float32)
        myeoh = np.zeros((1, E), dtype=np.float32)
        myeoh[0, j] = 1.0
        in_maps.append({
            "xst": np.ascontiguousarray(xflat[j * SLICE:(j + 1) * SLICE].T),
            "xf": xflat,
            "wr": wr,
            "w1": np.ascontiguousarray(w1[j]),
            "w2": np.ascontiguousarray(w2[j]),
            "b1t": np.ascontiguousarray(b1[j].reshape(H // 128, 128).T),
            "b2": b2[j].reshape(1, D),
            "tri": tri,
            "idn": idn,
            "myj": myj,
            "myeoh": myeoh,
        })
    return in_maps


def _assemble(results):
    out = np.zeros((T, D), dtype=np.float32)
    for j in range(NCORES):
        slot = results[j]["slot"]
        y = results[j]["yout"]
        tok = slot[:B * C, 0].astype(np.int64) - 1
        valid = tok >= 0
        out[tok[valid]] = y[valid]
    gs = results[0]["gstats"]          # [8, 20] identical on all cores
    counts = gs[:, 0:8]                # per slice per expert
    psums = gs[:, 8:16]
    z2 = gs[:, 16]
    density = (counts[0::2] + counts[1::2]) / float(N)       # [B, E]
    proxy = (psums[0::2] + psums[1::2]) / float(N)
    aux = np.float32(np.mean(np.sum(density * proxy, axis=-1)) * E * E)
    zl = np.float32(z2.sum() / float(T))
    return out.reshape(B, N, D), aux, zl


def kernel(**inputs):
    global _COMPILED
    from concourse.bass_utils import run_bass_kernel_spmd
    if _COMPILED is None:
        _COMPILED = _build()
    in_maps = _prepare_inputs(inputs)
    res = run_bass_kernel_spmd(_COMPILED, in_maps, list(range(NCORES)))
    return _assemble(res.results)


# revision 25
# speedup vs baseline: 1.0056x; 1.0005x over previous
"""MoE layer (top-1 switch routing) Trainium2 Bass kernel, expert-parallel over 8 cores.

Per core j (SPMD, same program, per-core data):
  Phase A: route its 1024-token slice (host-pretransposed xT, logits, softmax,
           argmax, in-slice cumsum via triangular matmul) -> per-token
           (idx, pos, gate) + per-slice stats -> table [1032,4] in DRAM.
  Phase B: AllGather tables across the 8 cores.
  Phase C: for all 8192 tokens compute this core's expert slot (global pos =
           slice pos + cross-slice carry, capacity clamp -> in-batch one-hot
           over 384 slots) and accumulate (token_id+1, gate) into PACKED
           per-m-tile PSUM via tiny permutation matmuls.  No indirect scatter.
  Phase D: indirect-gather the 1280 routed token rows of x, PE-transpose to
           xeT [d,1280] (f32r).
  Phase E: expert FFN y = gelu_tanh(xe@w1+b1)@w2+b2, f32r matmuls, H split in
           two halves with SBUF accumulation; epilogue scales rows by gate.
Host: inverse-permutation scatter of y rows into the output + aux/z losses
from the device-computed per-slice stats.
"""

import numpy as np

B, N, D, E, H = 4, 2048, 1024, 8, 4096
T = B * N                      # 8192 tokens
C = int(N * 1.25 // E)         # 320 expert capacity
NCORES = 8
SLICE = T // NCORES            # 1024 tokens routed per core
TT = SLICE // 128              # 8 token tiles per slice
M_TILES = (B * C) // 128       # 10 packed slot m-tiles (1280 rows)
TROWS = 1032                   # 1024 token rows + 5 stats rows + pad
S_PAD = 384                    # in-batch slot space (320 valid + trash at 383)
SLOT_ROWS = 1312

_COMPILED = None


def _pieces():
    """Packed-slot pieces: (b, cb0, m, p0, take): batch b's in-batch slots
    [cb0, cb0+take) land in packed m-tile m at partitions [p0, p0+take)."""
    out = []
    for b in range(B):
        c = 0
        while c < C:
            g = b * C + c
            m, p0 = divmod(g, 128)
            take = min(C - c, 128 - p0)
            out.append((b, c, m, p0, take))
            c += take
    return out


def _build():
    import concourse.bass as bass
    import concourse.bacc as bacc
    import concourse.mybir as mybir
    import concourse.tile as tile

    dt = mybir.dt
    AF = mybir.ActivationFunctionType
    OP = mybir.AluOpType

    nc = bacc.Bacc(trn_type="TRN2", num_devices=NCORES)

    xst_d = nc.dram_tensor("xst", [D, SLICE], dt.float32, kind="ExternalInput")
    xf_d = nc.dram_tensor("xf", [T, D], dt.float32, kind="ExternalInput")
    wr_d = nc.dram_tensor("wr", [D, E], dt.float32, kind="ExternalInput")
    w1_d = nc.dram_tensor("w1", [D, H], dt.float32r, kind="ExternalInput")
    w2_d = nc.dram_tensor("w2", [H, D], dt.float32r, kind="ExternalInput")
    b1t_d = nc.dram_tensor("b1t", [128, H // 128], dt.float32, kind="ExternalInput")
    b2_d = nc.dram_tensor("b2", [1, D], dt.float32, kind="ExternalInput")
    tri_d = nc.dram_tensor("tri", [128, 128], dt.float32, kind="ExternalInput")
    idn_d = nc.dram_tensor("idn", [128, 128], dt.float32, kind="ExternalInput")
    myj_d = nc.dram_tensor("myj", [128, 1], dt.float32, kind="ExternalInput")
    myeoh_d = nc.dram_tensor("myeoh", [1, E], dt.float32, kind="ExternalInput")

    yout_d = nc.dram_tensor("yout", [B * C, D], dt.float32, kind="ExternalOutput")
    slot_d = nc.dram_tensor("slot", [SLOT_ROWS, 2], dt.float32, kind="ExternalOutput")
    gstats_d = nc.dram_tensor("gstats", [NCORES, 20], dt.float32, kind="ExternalOutput")

    with tile.TileContext(nc) as tc:
        with tc.tile_pool(name="const", bufs=1) as cpool, \
             tc.tile_pool(name="dram", bufs=1, space="DRAM") as dpool:
            tri = cpool.tile([128, 128], dt.float32, tag="tri")
            idn = cpool.tile([128, 128], dt.float32, tag="idn")
            wr = cpool.tile([128, D // 128, E], dt.float32, tag="wr")
            myj = cpool.tile([128, 1], dt.float32, tag="myj")
            myeoh = cpool.tile([1, E], dt.float32, tag="myeoh")
            b1t = cpool.tile([128, H // 128], dt.float32, tag="b1t")
            b2r = cpool.tile([1, D], dt.float32, tag="b2r")
            ones_c = cpool.tile([128, 1], dt.float32, tag="ones_c")
            ones_r = cpool.tile([1, 128], dt.float32, tag="ones_r")
            iota_c = cpool.tile([128, 1], dt.float32, tag="iota_c")
            iota_e = cpool.tile([128, E], dt.float32, tag="iota_e")
            iota_s = cpool.tile([128, S_PAD], dt.float32, tag="iota_s")
            nc.sync.dma_start(out=tri[:], in_=tri_d[:])
            nc.sync.dma_start(out=idn[:], in_=idn_d[:])
            nc.sync.dma_start(out=wr[:], in_=wr_d[:].rearrange("(c p) e -> p c e", p=128))
            nc.sync.dma_start(out=myj[:], in_=myj_d[:])
            nc.sync.dma_start(out=myeoh[:], in_=myeoh_d[:])
            nc.sync.dma_start(out=b1t[:], in_=b1t_d[:])
            nc.sync.dma_start(out=b2r[:], in_=b2_d[:])
            nc.vector.memset(ones_c[:], 1.0)
            nc.vector.memset(ones_r[:], 1.0)
            ii = cpool.tile([128, 1], dt.int32, tag="ii")
            nc.gpsimd.iota(ii[:], pattern=[[0, 1]], base=0, channel_multiplier=1)
            nc.vector.tensor_copy(out=iota_c[:], in_=ii[:])
            ie = cpool.tile([128, E], dt.int32, tag="ie")
            nc.gpsimd.iota(ie[:], pattern=[[1, E]], base=0, channel_multiplier=0)
            nc.vector.tensor_copy(out=iota_e[:], in_=ie[:])
            iota_e3 = cpool.tile([128, 1, E], dt.float32, tag="iota_e3")
            nc.vector.tensor_copy(out=iota_e3[:], in_=iota_e[:])
            iota_em3 = cpool.tile([128, 1, E], dt.float32, tag="iota_em3")
            nc.vector.tensor_scalar_add(iota_em3[:], iota_e3[:], -100.0)
            it16 = cpool.tile([128, 2 * TT, 1], dt.int32, tag="it16")
            nc.gpsimd.iota(it16[:], pattern=[[1, 2 * TT], [0, 1]], base=0, channel_multiplier=0)
            ttg16f = cpool.tile([128, 2 * TT, 1], dt.float32, tag="ttg16f")
            nc.vector.tensor_copy(out=ttg16f[:], in_=it16[:])
            pb1 = cpool.tile([128, 2 * TT, 2], dt.bfloat16, tag="pb1")
            nc.vector.tensor_copy(out=pb1[:, :, 0:1],
                                  in_=iota_c[:, 0:1, None].to_broadcast([128, 2 * TT, 1]))
            nc.vector.memset(pb1[:, :, 1:2], 1.0)
            isl = cpool.tile([128, S_PAD], dt.int32, tag="isl")
            nc.gpsimd.iota(isl[:], pattern=[[1, S_PAD]], base=0, channel_multiplier=0)
            nc.vector.tensor_copy(out=iota_s[:], in_=isl[:])

            table = dpool.tile([TROWS, 4], dt.float32, tag="table")
            gtable = dpool.tile([NCORES * TROWS, 4], dt.float32, tag="gtable")

            # ---------------- Phase A: routing on own slice ----------------
            with tc.tile_pool(name="rsb", bufs=1) as rsb, \
                 tc.tile_pool(name="rps", bufs=2, space="PSUM") as rps, \
                 tc.tile_pool(name="rstat", bufs=1, space="PSUM") as rstat:
                xsT = rsb.tile([128, D // 128, SLICE], dt.float32, tag="xsT")
                for tt in range(TT):
                    nc.sync.dma_start(
                        out=xsT[:, :, tt * 128:(tt + 1) * 128],
                        in_=xst_d[:, tt * 128:(tt + 1) * 128].rearrange("(c p) t -> p c t", p=128))
                ps_tot = rstat.tile([1, E], dt.float32, space="PSUM", tag="ps_tot")
                ps_ps = rstat.tile([1, E], dt.float32, space="PSUM", tag="ps_ps")
                ps_z = rstat.tile([1, TT], dt.float32, space="PSUM", tag="ps_z")
                carry = rsb.tile([1, E], dt.float32, tag="carry")

                pl3 = rps.tile([128, TT, E], dt.float32, space="PSUM", tag="pl")
                for tt in range(TT):
                    for dd in range(D // 128):
                        nc.tensor.matmul(out=pl3[:, tt, :], lhsT=xsT[:, dd, tt * 128:(tt + 1) * 128],
                                         rhs=wr[:, dd, :], start=(dd == 0), stop=(dd == D // 128 - 1),
                                         skip_group_check=True)
                lg3 = rsb.tile([128, TT, E], dt.float32, tag="lg3")
                nc.vector.tensor_copy(out=lg3[:], in_=pl3[:])
                mx3 = rsb.tile([128, TT, 1], dt.float32, tag="mx3")
                nc.vector.tensor_reduce(out=mx3[:], in_=lg3[:], axis=mybir.AxisListType.X, op=OP.max)
                sh3 = rsb.tile([128, TT, E], dt.float32, tag="sh3")
                nc.vector.tensor_tensor(out=sh3[:], in0=lg3[:], in1=mx3[:].to_broadcast([128, TT, E]),
                                        op=OP.subtract)
                ex3 = rsb.tile([128, TT, E], dt.float32, tag="ex3")
                nc.scalar.activation(out=ex3[:], in_=sh3[:], func=AF.Exp)
                sume3 = rsb.tile([128, TT, 1], dt.float32, tag="sume3")
                nc.vector.tensor_reduce(out=sume3[:], in_=ex3[:], axis=mybir.AxisListType.X, op=OP.add)
                gate3 = rsb.tile([128, TT, 1], dt.float32, tag="gate3")
                nc.vector.reciprocal(gate3[:], sume3[:])
                lns3 = rsb.tile([128, TT, 1], dt.float32, tag="lns3")
                nc.scalar.activation(out=lns3[:], in_=sume3[:], func=AF.Ln)
                z3 = rsb.tile([128, TT, 1], dt.float32, tag="z3")
                nc.vector.tensor_add(z3[:], lns3[:], mx3[:])
                zsq3 = rsb.tile([128, TT, 1], dt.float32, tag="zsq3")
                nc.vector.tensor_mul(zsq3[:], z3[:], z3[:])
                mm3 = rsb.tile([128, TT, E], dt.float32, tag="mm3")
                nc.vector.tensor_tensor(out=mm3[:], in0=lg3[:], in1=mx3[:].to_broadcast([128, TT, E]),
                                        op=OP.is_equal)
                comb3 = rsb.tile([128, TT, E], dt.float32, tag="comb3")
                nc.vector.tensor_mul(comb3[:], mm3[:], iota_em3[:].to_broadcast([128, TT, E]))
                nc.vector.tensor_scalar_add(comb3[:], comb3[:], 100.0)
                idx3 = rsb.tile([128, TT, 1], dt.float32, tag="idx3")
                nc.vector.tensor_reduce(out=idx3[:], in_=comb3[:], axis=mybir.AxisListType.X, op=OP.min)
                oh3 = rsb.tile([128, TT, E], dt.float32, tag="oh3")
                nc.vector.tensor_tensor(out=oh3[:], in0=iota_e3[:].to_broadcast([128, TT, E]),
                                        in1=idx3[:].to_broadcast([128, TT, E]), op=OP.is_equal)
                probs3 = rsb.tile([128, TT, E], dt.float32, tag="probs3")
                nc.vector.tensor_mul(probs3[:], ex3[:], gate3[:].to_broadcast([128, TT, E]))

                pc3 = rps.tile([128, TT, E], dt.float32, space="PSUM", tag="pc")
                for tt in range(TT):
                    nc.tensor.matmul(out=pc3[:, tt, :], lhsT=tri[:], rhs=oh3[:, tt, :],
                                     start=True, stop=(tt == 0), skip_group_check=True)
                    if tt > 0:
                        nc.tensor.matmul(out=pc3[:, tt, :], lhsT=ones_r[:], rhs=carry[:],
                                         start=False, stop=True, skip_group_check=True)
                    nc.tensor.matmul(out=ps_tot[:], lhsT=ones_c[:], rhs=oh3[:, tt, :],
                                     start=(tt == 0), stop=(tt == TT - 1), skip_group_check=True)
                    if tt < TT - 1:
                        nc.vector.tensor_copy(out=carry[:], in_=ps_tot[:])
                for tt in range(TT):
                    nc.tensor.matmul(out=ps_ps[:], lhsT=ones_c[:], rhs=probs3[:, tt, :],
                                     start=(tt == 0), stop=(tt == TT - 1), skip_group_check=True)
                nc.tensor.matmul(out=ps_z[:], lhsT=ones_c[:], rhs=zsq3[:], start=True, stop=True)

                posw3 = rsb.tile([128, TT, E], dt.float32, tag="posw3")
                nc.vector.tensor_mul(posw3[:], pc3[:], oh3[:])
                pos3 = rsb.tile([128, TT, 1], dt.float32, tag="pos3")
                nc.vector.tensor_reduce(out=pos3[:], in_=posw3[:], axis=mybir.AxisListType.X, op=OP.add)
                nc.vector.tensor_scalar_add(pos3[:], pos3[:], -1.0)

                trow3 = rsb.tile([128, TT, 4], dt.float32, tag="trow3")
                nc.vector.tensor_copy(out=trow3[:, :, 0:1], in_=idx3[:])
                nc.vector.tensor_copy(out=trow3[:, :, 1:2], in_=pos3[:])
                nc.vector.tensor_copy(out=trow3[:, :, 2:3], in_=gate3[:])
                nc.vector.memset(trow3[:, :, 3:4], 0.0)
                nc.sync.dma_start(out=table[0:SLICE, :].rearrange("(t p) c -> p t c", p=128),
                                  in_=trow3[:])
                st = rsb.tile([1, 20], dt.float32, tag="st")
                nc.vector.tensor_copy(out=st[:, 0:8], in_=ps_tot[:])
                nc.vector.tensor_copy(out=st[:, 8:16], in_=ps_ps[:])
                zz8 = rsb.tile([1, TT], dt.float32, tag="zz8")
                nc.vector.tensor_copy(out=zz8[:], in_=ps_z[:])
                nc.vector.tensor_reduce(out=st[:, 16:17], in_=zz8[:], axis=mybir.AxisListType.X, op=OP.add)
                nc.vector.memset(st[:, 17:20], 0.0)
                tflat = table[:].rearrange("a b -> (a b)")
                nc.sync.dma_start(out=tflat[4096:4116], in_=st[:])

            # ---------------- Phase B: exchange ----------------
            nc.gpsimd.collective_compute(
                "AllGather", mybir.AluOpType.bypass,
                replica_groups=[list(range(NCORES))],
                ins=[table.opt()], outs=[gtable.opt()])

            gflat = gtable[:].rearrange("a b -> (a b)")
            with tc.tile_pool(name="gs", bufs=2) as gs:
                for s in range(NCORES):
                    gst = gs.tile([1, 20], dt.float32, tag="gst")
                    nc.sync.dma_start(out=gst[:], in_=gflat[s * 4128 + 4096: s * 4128 + 4116])
                    nc.sync.dma_start(out=gstats_d[s:s + 1, :], in_=gst[:])

            # ------------- Phase C: permutation into packed PSUM -------------
            pieces = _pieces()
            with tc.tile_pool(name="ffn", bufs=1) as ffn:
                xeT = [ffn.tile([128, B * C], dt.float32r, tag=f"xeT{d}", name=f"xeT{d}")
                       for d in range(D // 128)]
                gates = [ffn.tile([128, 1], dt.float32, tag=f"gate{m}", name=f"gate{m}")
                         for m in range(M_TILES)]
                ofsI = [ffn.tile([128, 1], dt.int32, tag=f"ofsI{m}", name=f"ofsI{m}")
                        for m in range(M_TILES)]
                pm = {}
                extracted = set()
                with tc.tile_pool(name="csb", bufs=3) as csb, \
                     tc.tile_pool(name="cps", bufs=1, space="PSUM") as cps, \
                     tc.tile_pool(name="ptp", bufs=2, space="PSUM") as ptp, \
                     tc.tile_pool(name="pmp", bufs=1, space="PSUM") as pmp:
                    for b in range(B):
                        bp = [p for p in pieces if p[0] == b]
                        for (_, _, m, _, _) in bp:
                            if m not in pm:
                                pm[m] = pmp.tile([3, 128], dt.float32, space="PSUM",
                                                 tag=f"pm{m % 5}", name=f"pm{m}")
                        trw3b = csb.tile([128, 2 * TT, 4], dt.float32, tag="trw3b")
                        for si, s_ in enumerate((2 * b, 2 * b + 1)):
                            nc.sync.dma_start(
                                out=trw3b[:, si * TT:(si + 1) * TT, :],
                                in_=gtable[s_ * TROWS:s_ * TROWS + SLICE, :].rearrange(
                                    "(t p) c -> p t c", p=128))
                        cbc3 = csb.tile([128, 2 * TT, 1], dt.float32, tag="cbc3")
                        nc.vector.memset(cbc3[:, 0:TT, :], 0.0)
                        ce = csb.tile([1, E], dt.float32, tag="ce")
                        nc.sync.dma_start(out=ce[:],
                                          in_=gflat[2 * b * 4128 + 4096: 2 * b * 4128 + 4104])
                        cem = csb.tile([1, E], dt.float32, tag="cem")
                        nc.vector.tensor_mul(cem[:], ce[:], myeoh[:])
                        cnt = csb.tile([1, 1], dt.float32, tag="cnt")
                        nc.vector.tensor_reduce(out=cnt[:], in_=cem[:],
                                                axis=mybir.AxisListType.X, op=OP.add)
                        pcb = cps.tile([128, 1], dt.float32, space="PSUM", tag="pcb")
                        nc.tensor.matmul(out=pcb[:], lhsT=ones_r[:], rhs=cnt[:],
                                         start=True, stop=True)
                        cbv = csb.tile([128, 1], dt.float32, tag="cbv")
                        nc.vector.tensor_copy(out=cbv[:], in_=pcb[:])
                        nc.vector.tensor_copy(out=cbc3[:, TT:2 * TT, :],
                                              in_=cbv[:, 0:1, None].to_broadcast([128, TT, 1]))
                        mine3 = csb.tile([128, 2 * TT, 1], dt.float32, tag="mine3")
                        nc.vector.tensor_tensor(out=mine3[:], in0=trw3b[:, :, 0:1],
                                                in1=myj[:, 0:1, None].to_broadcast([128, 2 * TT, 1]),
                                                op=OP.is_equal)
                        posf3 = csb.tile([128, 2 * TT, 1], dt.float32, tag="posf3")
                        nc.vector.tensor_tensor(out=posf3[:], in0=trw3b[:, :, 1:2],
                                                in1=cbc3[:], op=OP.add)
                        inb3 = csb.tile([128, 2 * TT, 1], dt.float32, tag="inb3")
                        nc.vector.tensor_scalar(inb3[:], posf3[:], float(C),
                                                scalar2=None, op0=OP.is_lt)
                        sel3 = csb.tile([128, 2 * TT, 1], dt.float32, tag="sel3")
                        nc.vector.tensor_mul(sel3[:], mine3[:], inb3[:])
                        dstb3 = csb.tile([128, 2 * TT, 1], dt.float32, tag="dstb3")
                        nc.vector.tensor_scalar_add(dstb3[:], posf3[:], float(-(S_PAD - 1)))
                        nc.vector.tensor_mul(dstb3[:], dstb3[:], sel3[:])
                        nc.vector.tensor_scalar_add(dstb3[:], dstb3[:], float(S_PAD - 1))
                        ohs3 = csb.tile([128, 2 * TT, S_PAD], dt.bfloat16, tag="ohs3")
                        nc.vector.tensor_tensor(out=ohs3[:],
                                                in0=iota_s[:, None, :].to_broadcast([128, 2 * TT, S_PAD]),
                                                in1=dstb3[:].to_broadcast([128, 2 * TT, S_PAD]),
                                                op=OP.is_equal)
                        tokg3 = csb.tile([128, 2 * TT, 3], dt.bfloat16, tag="tokg3")
                        cdv = csb.tile([128, 2 * TT, 1], dt.float32, tag="cdv")
                        nc.vector.tensor_scalar_add(cdv[:], ttg16f[:], float(16 * b))
                        nc.vector.tensor_copy(out=tokg3[:, :, 0:1], in_=cdv[:])
                        nc.vector.tensor_copy(out=tokg3[:, :, 1:3], in_=pb1[:])
                        for ttg in range(2 * TT):
                            first = (ttg == 0)
                            last = (ttg == 2 * TT - 1)
                            for (_, cb0, m, p0, take) in bp:
                                nc.tensor.matmul(out=pm[m][:, p0:p0 + take],
                                                 lhsT=tokg3[:, ttg, :],
                                                 rhs=ohs3[:, ttg, cb0:cb0 + take],
                                                 start=first, stop=last,
                                                 skip_group_check=True)
                        # extract completed m-tiles
                        for m in sorted(pm.keys()):
                            if m in extracted:
                                continue
                            if (m + 1) * 128 <= (b + 1) * C:
                                pmv = csb.tile([3, 128], dt.float32, tag="pmv")
                                nc.vector.tensor_copy(out=pmv[:], in_=pm[m][:])
                                pmt2 = ptp.tile([128, 3], dt.float32, space="PSUM", tag="pmt2")
                                nc.tensor.transpose(out=pmt2[:], in_=pmv[:],
                                                    identity=idn[0:3, 0:3])
                                slotv = csb.tile([128, 3], dt.float32, tag="slotv")
                                nc.vector.tensor_copy(out=slotv[:], in_=pmt2[:])
                                tokp1 = csb.tile([128, 1], dt.float32, tag="tokp1")
                                nc.vector.tensor_scalar_mul(tokp1[:], slotv[:, 0:1], 128.0)
                                nc.vector.tensor_add(tokp1[:], tokp1[:], slotv[:, 1:2])
                                nc.vector.tensor_add(tokp1[:], tokp1[:], slotv[:, 2:3])
                                tokf = csb.tile([128, 1], dt.float32, tag="tokf")
                                nc.vector.tensor_scalar_add(tokf[:], tokp1[:], -1.0)
                                nc.vector.tensor_copy(out=ofsI[m][:], in_=tokf[:])
                                # gather routing rows for the gate column
                                rsh = csb.tile([128, 1], dt.int32, tag="rsh")
                                nc.vector.tensor_scalar(rsh[:], ofsI[m][:], 10,
                                                        scalar2=None, op0=OP.arith_shift_right)
                                nc.vector.tensor_scalar(rsh[:], rsh[:], 8,
                                                        scalar2=None, op0=OP.mult)
                                grow = csb.tile([128, 1], dt.int32, tag="grow")
                                nc.vector.tensor_tensor(out=grow[:], in0=rsh[:], in1=ofsI[m][:],
                                                        op=OP.add)
                                gr4 = csb.tile([128, 4], dt.float32, tag="gr4")
                                nc.gpsimd.indirect_dma_start(
                                    out=gr4[:], out_offset=None, in_=gtable[:],
                                    in_offset=bass.IndirectOffsetOnAxis(ap=grow[:, 0:1], axis=0),
                                    bounds_check=NCORES * TROWS - 1, oob_is_err=False)
                                nc.vector.tensor_copy(out=gates[m][:], in_=gr4[:, 2:3])
                                slot2 = csb.tile([128, 2], dt.float32, tag="slot2")
                                nc.vector.tensor_copy(out=slot2[:, 0:1], in_=tokp1[:])
                                nc.vector.tensor_copy(out=slot2[:, 1:2], in_=gates[m][:])
                                nc.sync.dma_start(out=slot_d[m * 128:(m + 1) * 128, :], in_=slot2[:])
                                extracted.add(m)

                # ---------------- Phase D: gather + transpose ----------------
                with tc.tile_pool(name="dsb", bufs=4) as dsb, \
                     tc.tile_pool(name="dps", bufs=4, space="PSUM") as dps:
                    for m in range(M_TILES):
                        xe = dsb.tile([128, D], dt.float32, tag="xe")
                        nc.gpsimd.indirect_dma_start(
                            out=xe[:], out_offset=None, in_=xf_d[:],
                            in_offset=bass.IndirectOffsetOnAxis(ap=ofsI[m][:, 0:1], axis=0),
                            bounds_check=T - 1, oob_is_err=False)
                        for dd in range(D // 128):
                            pt = dps.tile([128, 128], dt.float32, space="PSUM", tag="sc")
                            nc.tensor.transpose(out=pt[:], in_=xe[:, dd * 128:(dd + 1) * 128],
                                                identity=idn[:])
                            nc.vector.tensor_copy(out=xeT[dd][:, m * 128:(m + 1) * 128], in_=pt[:])

                # ---------------- Phase E: FFN ----------------
                KH = 16                      # hh tiles per half
                _py_counter = [0]
                NCH = [(0, 512), (512, 512), (1024, 256)]
                y_acc = [ffn.tile([128, D], dt.float32, tag=f"yacc{m}", name=f"yacc{m}")
                         for m in range(M_TILES)]
                with tc.tile_pool(name="wsb", bufs=2) as wsb, \
                     tc.tile_pool(name="wp1", bufs=4) as wp1, \
                     tc.tile_pool(name="wp2", bufs=8) as wp2, \
                     tc.tile_pool(name="hsb", bufs=1) as hsb, \
                     tc.tile_pool(name="fps", bufs=3, space="PSUM") as fps, \
                     tc.tile_pool(name="lps", bufs=1, space="PSUM") as lps:
                    hT = [hsb.tile([128, B * C], dt.float32r, tag=f"hT{k}", name=f"hT{k}")
                          for k in range(KH)]
                    for half in range(2):
                        for hh in range(KH):
                            hg = half * KH + hh
                            w1t = wp1.tile([128, D // 128, 128], dt.float32r, tag="w1t")
                            nc.sync.dma_start(
                                out=w1t[:],
                                in_=w1_d[:, hg * 128:(hg + 1) * 128].rearrange("(c p) h -> p c h", p=128))
                            for (n0, nw) in NCH:
                                ph = fps.tile([128, 512], dt.float32, space="PSUM", tag="sc")
                                for kc in range(D // 128):
                                    nc.tensor.matmul(out=ph[:, :nw], lhsT=w1t[:, kc, :],
                                                     rhs=xeT[kc][:, n0:n0 + nw],
                                                     start=(kc == 0), stop=(kc == D // 128 - 1))
                                nc.scalar.activation(out=hT[hh][:, n0:n0 + nw], in_=ph[:, :nw],
                                                     func=AF.Gelu_apprx_tanh,
                                                     bias=b1t[:, hg:hg + 1], scale=1.0)
                        for mg in range(2):
                            ms = range(mg * 5, mg * 5 + 5)
                            for nh in range(2):
                                pys = {}
                                for m in ms:
                                    pys[m] = lps.tile([128, 512], dt.float32, space="PSUM",
                                                      tag=f"py{m % 5}", name=f"py{m % 5}")
                                for kk in range(KH):
                                    w2t = wp2.tile([128, 512], dt.float32r, tag="w2t")
                                    nc.sync.dma_start(
                                        out=w2t[:],
                                        in_=w2_d[(half * KH + kk) * 128:(half * KH + kk + 1) * 128,
                                                 nh * 512:(nh + 1) * 512])
                                    for m in ms:
                                        nc.tensor.matmul(out=pys[m][:],
                                                         lhsT=hT[kk][:, m * 128:(m + 1) * 128],
                                                         rhs=w2t[:], start=(kk == 0),
                                                         stop=(half == 0 and kk == KH - 1))
                                for m in ms:
                                    if half == 0:
                                        nc.vector.tensor_copy(out=y_acc[m][:, nh * 512:(nh + 1) * 512],
                                                              in_=pys[m][:])
                                    else:
                                        nc.tensor.matmul(out=pys[m][:], lhsT=ones_r[:],
                                                         rhs=b2r[:, nh * 512:(nh + 1) * 512],
                                                         start=False, stop=True)
                                        yf = wsb.tile([128, 512], dt.float32, tag="yf")
                                        nc.vector.tensor_add(yf[:], y_acc[m][:, nh * 512:(nh + 1) * 512],
                                                             pys[m][:])
                                        nc.vector.tensor_scalar_mul(yf[:], yf[:], gates[m][:, 0:1])
                                        nc.sync.dma_start(
                                            out=yout_d[m * 128:(m + 1) * 128, nh * 512:(nh + 1) * 512],
                                            in_=yf[:])

    nc.compile()
    return nc


def _prepare_inputs(inputs):
    x = np.asarray(inputs["x"], dtype=np.float32)
    wr = np.asarray(inputs["w_router"], dtype=np.float32)
    w1 = np.asarray(inputs["w1"], dtype=np.float32)
    b1 = np.asarray(inputs["b1"], dtype=np.float32)
    w2 = np.asarray(inputs["w2"], dtype=np.float32)
    b2 = np.asarray(inputs["b2"], dtype=np.float32)
    xflat = np.ascontiguousarray(x.reshape(T, D))
    tri = np.triu(np.ones((128, 128), dtype=np.float32))
    idn = np.eye(128, dtype=np.float32)
    in_maps = []
    for j in range(NCORES):
        myj = np.full((128, 1), float(j), dtype=np.float32)
        myeoh = np.zeros((1, E), dtype=np.float32)
        myeoh[0, j] = 1.0
        in_maps.append({
            "xst": np.ascontiguousarray(xflat[j * SLICE:(j + 1) * SLICE].T),
            "xf": xflat,
            "wr": wr,
            "w1": np.ascontiguousarray(w1[j]),
            "w2": np.ascontiguousarray(w2[j]),
            "b1t": np.ascontiguousarray(b1[j].reshape(H // 128, 128).T),
            "b2": b2[j].reshape(1, D),
            "tri": tri,
            "idn": idn,
            "myj": myj,
            "myeoh": myeoh,
        })
    return in_maps


def _assemble(results):
    out = np.zeros((T, D), dtype=np.float32)
    for j in range(NCORES):
        slot = results[j]["slot"]
        y = results[j]["yout"]
        tok = slot[:B * C, 0].astype(np.int64) - 1
        valid = tok >= 0
        out[tok[valid]] = y[valid]
    gs = results[0]["gstats"]          # [8, 20] identical on all cores
    counts = gs[:, 0:8]                # per slice per expert
    psums = gs[:, 8:16]
    z2 = gs[:, 16]
    density = (counts[0::2] + counts[1::2]) / float(N)       # [B, E]
    proxy = (psums[0::2] + psums[1::2]) / float(N)
    aux = np.float32(np.mean(np.sum(density * proxy, axis=-1)) * E * E)
    zl = np.float32(z2.sum() / float(T))
    return out.reshape(B, N, D), aux, zl


def kernel(**inputs):
    global _COMPILED
    from concourse.bass_utils import run_bass_kernel_spmd
    if _COMPILED is None:
        _COMPILED = _build()
    in_maps = _prepare_inputs(inputs)
    res = run_bass_kernel_spmd(_COMPILED, in_maps, list(range(NCORES)))
    return _assemble(res.results)
